# revision 33
# baseline (speedup 1.0000x reference)
"""Distributed causal multi-head attention for 8 TRN2 NeuronCores.

Problem: x[4,2048,512], 8 heads, causal. out = Attn(x) @ Wo.T + bo.

Sharding: 2 cores per batch element, split by HEADS (Megatron-style):
core (b,h) computes heads [4h, 4h+4) for ALL 2048 query rows of batch b.
Each core runs the identical SPMD graph; all per-core differences flow
through input data (weight slices, per-half bo). The O-projection
contracts only this core's 4 heads, so each core emits a PARTIAL output
[2048, 512] (bf16) and the host sums the two halves per batch.

vs the query-row split this kernel replaces: the K/V projections are no
longer computed twice per core pair (-27us of PE streams per core), the
Q projection reads x^T directly (no gathered copy), weight DMA halves,
and the causal-triangle load balance across the core pair is exact by
construction.

Device layouts are transposed so per-query softmax reductions become
matmuls / ones-column tricks instead of partition reductions:
  QT[j,q], KT[j,k] from  W.T @ x.T ;  V[k,j] natural;
  S^T[k,q] = KT_head.T @ QT_head (heads on partitions 0-63);
  P = exp(S^T/8) on ScalarE, causal mask as a DVE multiply (2x mode);
  o^T[d,q] accumulated over k-blocks with two heads col-packed in PSUM;
  softmax denominator rides along as a ones-column of V.

Performance structure (vs 153us baseline):
  - 8 query panels of 256 rows with TIGHT k-extents: panel j attends
    k < 256(j+1) exactly (2(j+1) 128-blocks), no 512-padding. Only the
    last 2 k-blocks of each panel are triangular; the mask tensor is
    panel-independent ([P,2,2,QP], 256KB instead of 2.1MB).
  - K^T/Q^T head-PAIR-stacked on 128 partitions: the two heads' K=64
    score matmuls run concurrently as PE row tiles (0,0)/(64,0). Their
    outputs land in different PSUM banks (hh-major s layout) -
    concurrent same-bank drains are device-fatal.
  - all DRAM inputs pre-packed host-side so every dma_start is a fully
    contiguous [128, bytes] block; wk / x chunk 0 split by contraction
    block so the first matmul starts ~2us in.
  - chunk-pipelined: projection matmul groups for k-chunk c are emitted
    between attention batches of panel c (c=1..3), so PE always has
    dense ready work while ScalarE runs exp.
  - K bias dropped (adds a per-q constant to scores -> cancels in
    softmax); V bias folded host-side into bo' = bo + bv_h @ Wo_h.T.
  - head-split of K/Q projections via DVE copies out of PSUM; V copies
    on ScalarE to balance engine load.
  - the normalize chain (den copy -> recip -> gpsimd partition_broadcast
    of 1/den -> muls) of pair pr is emitted inside pair pr+1's score
    stream, so the in-order PE queue never head-of-line blocks on the
    DVE chain. reciprocal_approx_fast must read SBUF, not PSUM (PSUM
    input returns garbage on HW even though CoreSim accepts it).
  - O-projection contracts the 2 head pairs (K=128 each), deferred
    behind the next panel's scores; the last panel accumulates it per
    head pair so the kernel tail is one matmul pair.
  - fp8 was evaluated and REJECTED: e4m3 quantization anywhere in the
    V/p/att/Wo path costs >=2.2e-2 rel err alone (errors pass through
    attention averaging undiminished); tolerance is 2e-2.
"""

import os
import sys

import numpy as np

sys.path.insert(0, "/opt/trn_rl_repo")

import concourse.bass as bass  # noqa: E402
import concourse.mybir as mybir  # noqa: E402
from concourse import bacc, library_config  # noqa: E402
from concourse.tile import TileContext  # noqa: E402

P = 128
D = 512
S = 2048
HLOC = 4  # heads per core
DH = 64
NPANEL = 8
QP = 256  # query rows per panel
JW = HLOC * DH  # local j-width of K/Q/V projections (256)
SCALE = 0.125  # 1/sqrt(DH)

MMDT_NAME = os.environ.get("KERNEL_MMDT", "bf16")
MASK_GS = os.environ.get("KERNEL_MASK_GS", "0") == "1"
PVDEPTH = int(os.environ.get("KERNEL_PVDEPTH", "4"))
VCOPY_ACT = os.environ.get("KERNEL_VCOPY_ACT", "0") == "1"

f32 = mybir.dt.float32
Exp = mybir.ActivationFunctionType.Exp
add_op = mybir.AluOpType.add
mult_op = mybir.AluOpType.mult

MMDT = {"bf16": mybir.dt.bfloat16, "f32r": mybir.dt.float32r, "f32": f32}[MMDT_NAME]


def build():
    # Bacc (not Bass): its compile() pipeline runs generate_event_semaphores,
    # which splits multi-wait instructions to satisfy the 1-wait-per-
    # instruction hardware limit.
    nc = bacc.Bacc()

    xTp = nc.declare_dram_parameter("xTp", [P, 4, 4, 512], MMDT, isOutput=False)
    wkp = nc.declare_dram_parameter("wkp", [P, 4, JW], MMDT, isOutput=False)
    wqp = nc.declare_dram_parameter("wqp", [P, 4, JW], MMDT, isOutput=False)
    wvp = nc.declare_dram_parameter("wvp", [P, 4, JW], MMDT, isOutput=False)
    wop = nc.declare_dram_parameter("wop", [P, 2, D], MMDT, isOutput=False)
    bq = nc.declare_dram_parameter("bq", [P, 2], f32, isOutput=False)
    bo_bc = nc.declare_dram_parameter("bo_bc", [P, D], f32, isOutput=False)
    maskp = nc.declare_dram_parameter("maskp", [P, 2, 2, QP], MMDT, isOutput=False)
    # [panel, partition, q-subtile, d]: lets each panel's output leave in a
    # single contiguous dma_start (the host un-permutes)
    out = nc.declare_dram_parameter("out", [NPANEL, P, 2, D], MMDT, isOutput=True)

    with nc.allow_low_precision(reason="bf16 matmul operands"), TileContext(nc) as tc:
        with (
            tc.tile_pool(name="big", bufs=1) as bpool,
            tc.tile_pool(name="attp", bufs=2) as apool,
            tc.tile_pool(name="work", bufs=PVDEPTH + 2) as wpool,
            tc.tile_pool(name="osb", bufs=2) as opool,
            tc.tile_pool(name="ps_proj", bufs=2, space="PSUM") as ps_proj,
            tc.tile_pool(name="ps_s", bufs=2, space="PSUM") as ps_s,
            tc.tile_pool(name="ps_ot", bufs=2, space="PSUM") as ps_ot,
        ):
            # ---- persistent SBUF tensors ----
            xT_sb = bpool.tile([P, 4, 4, 512], MMDT, tag="xT")
            # K^T/Q^T head-PAIR-stacked: head 2pr on partitions 0-63,
            # head 2pr+1 on 64-127.
            kT_sb = bpool.tile([P, 2, S], MMDT, tag="kT")
            qT_sb = bpool.tile([P, 2, S], MMDT, tag="qT")
            v_sb = bpool.tile([P, S // P, HLOC, DH + 1], MMDT, tag="v")
            wk_sb = bpool.tile([P, 4, JW], MMDT, tag="wk", name="wk")
            wq_sb = bpool.tile([P, 4, JW], MMDT, tag="wq", name="wq")
            wv_sb = bpool.tile([P, 4, JW], MMDT, tag="wv", name="wv")
            wo_sb = bpool.tile([P, 2, D], MMDT, tag="wo")
            bq_sb = bpool.tile([P, 2], f32, tag="bq")
            bo_sb = bpool.tile([P, D], f32, tag="bo")
            mask_sb = bpool.tile([P, 2, 2, QP], MMDT, tag="mask")

            # input DMAs in consumption order; every transfer is contiguous.
            # Each dma_start costs ~0.6us of Sync-engine descriptor issue, so
            # transfers are batched as large as dependency order allows.
            nc.sync.dma_start(out=wk_sb[:], in_=wkp[:])
            nc.sync.dma_start(out=xT_sb[:, 0, 0:2], in_=xTp[:, 0, 0:2])
            nc.sync.dma_start(out=xT_sb[:, 0, 2:4], in_=xTp[:, 0, 2:4])
            nc.sync.dma_start(out=wq_sb[:], in_=wqp[:])
            nc.sync.dma_start(out=bq_sb[:], in_=bq[:])
            nc.sync.dma_start(out=mask_sb[:], in_=maskp[:])
            nc.sync.dma_start(out=wv_sb[:], in_=wvp[:])
            nc.sync.dma_start(out=xT_sb[:, 1], in_=xTp[:, 1])
            nc.sync.dma_start(out=wo_sb[:], in_=wop[:])
            nc.sync.dma_start(out=bo_sb[:], in_=bo_bc[:])
            nc.sync.dma_start(out=xT_sb[:, 2:4], in_=xTp[:, 2:4])
            # ones column appended per head so P.V also yields the softmax
            # denominator in psum row DH for free
            nc.vector.memset(v_sb[:, :, :, DH : DH + 1], 1.0)
            # gpsimd runs the 1/den partition broadcast (attn library)
            nc.gpsimd.load_library(library_config.attn)

            def proj_kq_gen(kc):
                """Yields after each matmul group so the caller can
                interleave projection work into the attention stream."""
                # K^T[j, k-chunk]; no bias (cancels in softmax)
                for pr in range(2):
                    ps = ps_proj.tile([P, 512], f32, tag="p512")
                    for db in range(4):
                        nc.tensor.matmul(
                            ps[:],
                            lhsT=wk_sb[:, db, pr * P : (pr + 1) * P],
                            rhs=xT_sb[:, kc, db, :],
                            start=(db == 0),
                            stop=(db == 3),
                        )
                    # pure cast -> ScalarE (DVE is the busier drain engine)
                    nc.scalar.copy(
                        out=kT_sb[:, pr, kc * 512 : (kc + 1) * 512], in_=ps[:]
                    )
                    yield
                # Q^T for q-chunk kc (panels 2kc, 2kc+1)
                for pr in range(2):
                    ps = ps_proj.tile([P, 512], f32, tag="p512")
                    for db in range(4):
                        nc.tensor.matmul(
                            ps[:],
                            lhsT=wq_sb[:, db, pr * P : (pr + 1) * P],
                            rhs=xT_sb[:, kc, db, :],
                            start=(db == 0),
                            stop=(db == 3),
                        )
                    nc.vector.tensor_tensor(
                        qT_sb[:, pr, kc * 512 : (kc + 1) * 512],
                        ps[:],
                        bq_sb[:, pr : pr + 1].to_broadcast([P, 512]),
                        add_op,
                    )
                    yield

            def proj_v_gen(kc):
                # V[k-chunk, j]; no bias (folded into bo' host-side)
                for kb in range(4):
                    ps = ps_proj.tile([P, 512], f32, tag="p512")
                    pv = ps[:, 0:JW]
                    for db in range(4):
                        nc.tensor.matmul(
                            pv,
                            lhsT=xT_sb[:, kc, db, kb * P : (kb + 1) * P],
                            rhs=wv_sb[:, db, :],
                            start=(db == 0),
                            stop=(db == 3),
                        )
                    # early chunks land while ACT is exp-light; later chunks
                    # drain on DVE to keep the exp stream unobstructed
                    if VCOPY_ACT or kc <= 1:
                        nc.scalar.copy(
                            out=v_sb[:, 4 * kc + kb, :, 0:DH],
                            in_=pv.rearrange("p (h d) -> p h d", h=HLOC),
                        )
                    else:
                        nc.vector.tensor_copy(
                            out=v_sb[:, 4 * kc + kb, :, 0:DH],
                            in_=pv.rearrange("p (h d) -> p h d", h=HLOC),
                        )
                    yield

            def make_norm(pr, ot_ps, attT_sb, split_hh=False, den_act=False):
                def emit_norm():
                    # attT[:, pr, :] = ot / den; den sits in psum row DH.
                    # custom-DVE recip must read SBUF (PSUM input returns
                    # garbage on HW even though CoreSim accepts it).
                    den_sb = wpool.tile([1, 2, QP], f32, tag="den_sb")
                    rden_f = wpool.tile([1, 2, QP], f32, tag="rden_f")
                    bc_sb = wpool.tile([DH, 2, QP], f32, tag="bc_sb")
                    if split_hh:
                        # per-hh chains pipeline across DVE/GpSimd: lower
                        # latency; den_act puts the PSUM drain on ScalarE
                        # (idle at the kernel tail) so DVE starts recip sooner
                        for hh in range(2):
                            if den_act:
                                nc.scalar.copy(
                                    out=den_sb[:, hh, :],
                                    in_=ot_ps[DH : DH + 1, hh, :],
                                )
                            else:
                                nc.vector.tensor_copy(
                                    out=den_sb[:, hh, :],
                                    in_=ot_ps[DH : DH + 1, hh, :],
                                )
                            nc.vector.reciprocal_approx_fast(
                                out=rden_f[:, hh, :], in_=den_sb[:, hh, :]
                            )
                            nc.gpsimd.partition_broadcast(
                                bc_sb[:, hh, :], rden_f[:, hh, :]
                            )
                    else:
                        nc.vector.tensor_copy(
                            out=den_sb[:], in_=ot_ps[DH : DH + 1, :, :]
                        )
                        nc.vector.reciprocal_approx_fast(
                            out=rden_f[:], in_=den_sb[:]
                        )
                        # broadcast 1/den across the 64 dh partitions on
                        # gpsimd (keeps PE/DVE out of the norm critical path)
                        nc.gpsimd.partition_broadcast(bc_sb[:], rden_f[:])
                    for hh in range(2):
                        nc.vector.tensor_mul(
                            out=attT_sb[hh * DH : (hh + 1) * DH, pr, :],
                            in0=ot_ps[0:DH, hh, :],
                            in1=bc_sb[:, hh, :],
                        )

                return emit_norm

            # last panel: O-projection accumulates per head pair as each
            # norm completes, so the kernel tail is one matmul pair
            last_ps = {}

            def make_o_mm(p, pr, attT_sb, start, stop):
                def emit():
                    osb = None
                    for qs in range(2):
                        if start:
                            last_ps[qs] = ps_proj.tile(
                                [P, D], f32, tag="p512", name=f"lastps{qs}"
                            )
                        nc.tensor.matmul(
                            last_ps[qs][:],
                            lhsT=attT_sb[:, pr, qs * P : (qs + 1) * P],
                            rhs=wo_sb[:, pr, :],
                            start=start,
                            stop=stop,
                        )
                        if stop:
                            if osb is None:
                                osb = opool.tile([P, 2, D], MMDT, tag="osb")
                            nc.vector.tensor_tensor(
                                osb[:, qs, :], last_ps[qs][:], bo_sb[:], add_op
                            )
                    if stop:
                        nc.sync.dma_start(out=out[p], in_=osb[:])

                return emit

            def make_oproj(p, attT_sb):
                def emit_oproj():
                    # out[q,:] = attT.T @ Wo_h.T + bo'; the two head pairs
                    # contract 128 partitions each
                    osb = opool.tile([P, 2, D], MMDT, tag="osb")
                    for qs in range(2):
                        ps = ps_proj.tile([P, D], f32, tag="p512")
                        for pr in range(2):
                            nc.tensor.matmul(
                                ps[:],
                                lhsT=attT_sb[:, pr, qs * P : (qs + 1) * P],
                                rhs=wo_sb[:, pr, :],
                                start=(pr == 0),
                                stop=(pr == 1),
                            )
                        nc.vector.tensor_tensor(
                            osb[:, qs, :], ps[:], bo_sb[:], add_op
                        )
                    nc.sync.dma_start(out=out[p], in_=osb[:])

                return emit_oproj

            # deferred work from panel p-1, emitted at staggered slots inside
            # panel p's batch stream so norm chains / O-projections hide
            # behind dense score+PV work. Entries are (min_slot, fn); the
            # PE-visible O-proj goes late (its matmuls wait on attT, and the
            # in-order PE queue would head-of-line block everything emitted
            # after it).
            deferred_q = []
            # PV software pipeline, GLOBAL across panels: each slot first
            # emits the PV queued PVDEPTH slots ago (before its own scores,
            # so at panel boundaries the PE has ready work in front of the
            # next panel's still-waiting score matmuls), then queues its own.
            # Entries are (emit_fn, after_fn): after_fn fires the pair's
            # norm chain as soon as that pair's last PV is emitted.
            pending = []

            def pop_pending():
                fn, after = pending.pop(0)
                fn()
                if after is not None:
                    after()

            def emit_attention_panel(p, gen, drain_gen=True, rate=1):
                nbat = p + 1  # 2 k-blocks per exp batch, k < 256(p+1)
                q0 = p * QP
                attT_sb = apool.tile([P, 2, QP], MMDT, tag="attT")
                ot_ps = [
                    ps_ot.tile([DH + 1, 2, QP], f32, tag="ot", name=f"ot{pr}")
                    for pr in range(2)
                ]

                def emit_pv(bb, pr, pT):
                    for kbi in range(2):
                        for hh in range(2):
                            h = 2 * pr + hh
                            nc.tensor.matmul(
                                ot_ps[pr][:, hh, :],
                                lhsT=v_sb[:, 2 * bb + kbi, h, :],
                                rhs=pT[:, hh, kbi, :],
                                start=(bb == 0 and kbi == 0 and hh == 0),
                                stop=(bb == nbat - 1 and kbi == 1 and hh == 1),
                            )

                # the two head pairs' batches are INTERLEAVED (pr inner) so
                # each pair's norm chain hides behind the other pair's dense
                # score/PV stream; PV for slot s-PVDEPTH is emitted after
                # scores+exp of slot s, so the in-order PE queue always has
                # ready matmuls while ACT runs the exp
                # the LAST panel runs pr1 before pr0 each batch so pr1's
                # norm chain and O-matmuls overlap pr0's final PV stream
                last = p == NPANEL - 1
                pr_order = (1, 0) if last else (0, 1)
                slot = 0
                for bb in range(nbat):
                    for pr in pr_order:
                        if len(pending) >= PVDEPTH:
                            pop_pending()
                        s_ps = ps_s.tile([P, 2, 2, QP], f32, tag="s")
                        for kbi in range(2):
                            kb = 2 * bb + kbi
                            for hh in range(2):
                                nc.tensor.matmul(
                                    s_ps[:, hh, kbi, :],
                                    lhsT=kT_sb[
                                        hh * DH : (hh + 1) * DH,
                                        pr,
                                        kb * P : (kb + 1) * P,
                                    ],
                                    rhs=qT_sb[
                                        hh * DH : (hh + 1) * DH, pr, q0 : q0 + QP
                                    ],
                                    start=True,
                                    stop=True,
                                )
                        pT = wpool.tile([P, 2, 2, QP], MMDT, tag="pT")
                        nc.scalar.activation(pT[:], s_ps[:], Exp, scale=SCALE)
                        if bb == nbat - 1:
                            # zero masked probabilities in the 2 triangular
                            # k-blocks; mask pre-expanded over the head dim ->
                            # no broadcast operand -> DVE 2x mode
                            for kbi in range(2):
                                if MASK_GS:
                                    eng = nc.gpsimd if kbi == 1 else nc.vector
                                else:
                                    eng = nc.vector
                                eng.tensor_tensor(
                                    pT[:, :, kbi, :],
                                    pT[:, :, kbi, :],
                                    mask_sb[:, kbi, :, :],
                                    mult_op,
                                )
                        after = None
                        if bb == nbat - 1:
                            after = make_norm(
                                pr,
                                ot_ps[pr],
                                attT_sb,
                                split_hh=True,
                                den_act=last,
                            )
                        pending.append(
                            (
                                lambda bb=bb, pr=pr, pT=pT: emit_pv(bb, pr, pT),
                                after,
                            )
                        )
                        while deferred_q and deferred_q[0][0] <= slot:
                            deferred_q.pop(0)[1]()
                        # keep PE dense: pull next projection group for the
                        # following k-chunk while ACT digests this batch
                        if gen is not None and slot % rate == 0:
                            next(gen, None)
                        slot += 1
                if last:
                    # kernel tail: drain everything; each pair's norm chain
                    # fires with its last PV, then pr1's O-matmuls run on PE
                    # while DVE/GpSimd still normalize pr0
                    while pending:
                        pop_pending()
                    while deferred_q:
                        deferred_q.pop(0)[1]()
                    make_o_mm(p, pr_order[0], attT_sb, start=True, stop=False)()
                    make_o_mm(p, pr_order[1], attT_sb, start=False, stop=True)()
                else:
                    # leftover PVs spill into the next panel's early slots;
                    # flush any stale deferred O-proj before queueing ours
                    while deferred_q:
                        deferred_q.pop(0)[1]()
                    deferred_q.append((5, make_oproj(p, attT_sb)))
                # drain any leftover projection groups of the next chunk
                if gen is not None and drain_gen:
                    for _ in gen:
                        pass

            def proj_chunk_gen(kc):
                yield from proj_kq_gen(kc)
                yield from proj_v_gen(kc)

            # chunk-0 K/Q go first (panel 0's scores need them); its V
            # groups interleave into panel 0 so the first exp starts sooner.
            # chunk 1 feeds panel 2 (Q) / panel 3 (K,V); chunks 2-3 are
            # spread across two panels each at half rate so the late panels'
            # boundaries still have independent PE work to hide norm chains
            for _ in proj_kq_gen(0):
                pass
            gens = {1: proj_chunk_gen(1), 2: proj_chunk_gen(2), 3: proj_chunk_gen(3)}
            sched = {
                0: (proj_v_gen(0), True, 1),
                1: (gens[1], True, 1),
                2: (gens[2], False, 2),
                3: (gens[2], True, 2),
                4: (gens[3], False, 2),
                5: (gens[3], True, 2),
            }
            for p in range(NPANEL):
                gen, drain, rate = sched.get(p, (None, True, 1))
                emit_attention_panel(p, gen, drain, rate)
    return nc


_NC = None


def _get_nc():
    global _NC
    if _NC is None:
        _NC = build()
        # run_bass_via_pjrt does not finalize; Bacc.finalize runs the compile
        # passes (register allocation, event-semaphore wait splitting).
        _NC.finalize()
    return _NC


def _mask_tri(mmnp):
    # triangular masks for the last 2 k-blocks of every panel; panel-
    # independent: block i valid where i*128 + r <= c
    m = np.empty((P, 2, 2, QP), np.float32)
    r = np.arange(P)[:, None]
    c = np.arange(QP)[None, :]
    for i in range(2):
        mi = np.where(i * P + r <= c, 1.0, 0.0)
        m[:, i, 0, :] = mi
        m[:, i, 1, :] = mi
    return np.ascontiguousarray(m).astype(mmnp)


def _in_maps(inputs):
    mmnp = mybir.dt.np(MMDT)
    x = np.asarray(inputs["x"], np.float32)
    woT = np.asarray(inputs["W_O_w"], np.float32).T  # [(h,dh), n]
    bo_full = np.asarray(inputs["W_O_b"], np.float32)
    bv_full = np.asarray(inputs["W_V_b"], np.float32)
    bq_full = np.asarray(inputs["W_Q_b"], np.float32)
    mask = _mask_tri(mmnp)
    halves = []
    for h in range(2):
        jsl = slice(JW * h, JW * (h + 1))
        # [P, db, j]: contraction row d = db*128 + p
        wq = np.ascontiguousarray(
            np.asarray(inputs["W_Q_w"], np.float32)[jsl].T.reshape(4, P, JW)
            .transpose(1, 0, 2)
        ).astype(mmnp)
        wk = np.ascontiguousarray(
            np.asarray(inputs["W_K_w"], np.float32)[jsl].T.reshape(4, P, JW)
            .transpose(1, 0, 2)
        ).astype(mmnp)
        wv = np.ascontiguousarray(
            np.asarray(inputs["W_V_w"], np.float32)[jsl].T.reshape(4, P, JW)
            .transpose(1, 0, 2)
        ).astype(mmnp)
        woT_h = woT[jsl]  # [256, 512]
        wo = np.ascontiguousarray(
            woT_h.reshape(2, P, D).transpose(1, 0, 2)
        ).astype(mmnp)
        bqh = np.ascontiguousarray(bq_full[jsl].reshape(2, P).T)
        # V bias folded through the O projection; bo itself only on half 0
        # (the two halves' partial outputs are summed on the host)
        bo_eff = (bo_full if h == 0 else 0.0) + bv_full[jsl] @ woT_h
        bo_bc = np.ascontiguousarray(np.broadcast_to(bo_eff, (P, D)))
        halves.append(dict(wqp=wq, wkp=wk, wvp=wv, wop=wo, bq=bqh, bo_bc=bo_bc))
    in_maps = []
    for core in range(8):
        b, h = core // 2, core % 2
        xT = x[b].T  # [D, S]
        xTp = np.ascontiguousarray(
            xT.reshape(4, P, 4, 512).transpose(1, 2, 0, 3)
        ).astype(mmnp)
        m = dict(halves[h])
        m["xTp"] = xTp
        m["maskp"] = mask
        in_maps.append(m)
    return in_maps


def _assemble(results, B=4):
    out = np.empty((B, S, D), np.float32)
    for b in range(B):
        # device layout [panel, p, qs, d] -> q = 256*panel + 128*qs + p
        s = results[2 * b]["out"].astype(np.float32) + results[2 * b + 1][
            "out"
        ].astype(np.float32)
        out[b] = s.reshape(NPANEL, P, 2, D).transpose(0, 2, 1, 3).reshape(S, D)
    return out


def run(inputs, trace=False, **kw):
    from concourse.bass_utils import run_bass_kernel_spmd

    res = run_bass_kernel_spmd(
        _get_nc(), _in_maps(inputs), core_ids=list(range(8)), trace=trace, **kw
    )
    return _assemble(res.results), res


def kernel(**inputs):
    out, _ = run(inputs, trace=False)
    return out


# revision 35
# speedup vs baseline: 1.0142x; 1.0142x over previous
"""Distributed causal multi-head attention for 8 TRN2 NeuronCores.

Problem: x[4,2048,512], 8 heads, causal. out = Attn(x) @ Wo.T + bo.

Sharding: 2 cores per batch element, split by HEADS (Megatron-style):
core (b,h) computes heads [4h, 4h+4) for ALL 2048 query rows of batch b.
Each core runs the identical SPMD graph; all per-core differences flow
through input data (weight slices, per-half bo). The O-projection
contracts only this core's 4 heads, so each core emits a PARTIAL output
[2048, 512] (bf16) and the host sums the two halves per batch.

vs the query-row split this kernel replaces: the K/V projections are no
longer computed twice per core pair (-27us of PE streams per core), the
Q projection reads x^T directly (no gathered copy), weight DMA halves,
and the causal-triangle load balance across the core pair is exact by
construction.

Device layouts are transposed so per-query softmax reductions become
matmuls / ones-column tricks instead of partition reductions:
  QT[j,q], KT[j,k] from  W.T @ x.T ;  V[k,j] natural;
  S^T[k,q] = KT_head.T @ QT_head (heads on partitions 0-63);
  P = exp(S^T/8) on ScalarE, causal mask as a DVE multiply (2x mode);
  o^T[d,q] accumulated over k-blocks with two heads col-packed in PSUM;
  softmax denominator rides along as a ones-column of V.

Performance structure (vs 153us baseline):
  - 8 query panels of 256 rows with TIGHT k-extents: panel j attends
    k < 256(j+1) exactly (2(j+1) 128-blocks), no 512-padding. Only the
    last 2 k-blocks of each panel are triangular; the mask tensor is
    panel-independent ([P,2,2,QP], 256KB instead of 2.1MB).
  - K^T/Q^T head-PAIR-stacked on 128 partitions: the two heads' K=64
    score matmuls run concurrently as PE row tiles (0,0)/(64,0). Their
    outputs land in different PSUM banks (hh-major s layout) -
    concurrent same-bank drains are device-fatal.
  - all DRAM inputs pre-packed host-side so every dma_start is a fully
    contiguous [128, bytes] block; wk / x chunk 0 split by contraction
    block so the first matmul starts ~2us in.
  - chunk-pipelined: projection matmul groups for k-chunk c are emitted
    between attention batches of panel c (c=1..3), so PE always has
    dense ready work while ScalarE runs exp.
  - K bias dropped (adds a per-q constant to scores -> cancels in
    softmax); V bias folded host-side into bo' = bo + bv_h @ Wo_h.T.
  - head-split of K/Q projections via DVE copies out of PSUM; V copies
    on ScalarE to balance engine load.
  - the normalize chain (den copy -> recip -> gpsimd partition_broadcast
    of 1/den -> muls) of pair pr is emitted inside pair pr+1's score
    stream, so the in-order PE queue never head-of-line blocks on the
    DVE chain. reciprocal_approx_fast must read SBUF, not PSUM (PSUM
    input returns garbage on HW even though CoreSim accepts it).
  - O-projection contracts the 2 head pairs (K=128 each), deferred
    behind the next panel's scores; the last panel accumulates it per
    head pair so the kernel tail is one matmul pair.
  - fp8 was evaluated and REJECTED: e4m3 quantization anywhere in the
    V/p/att/Wo path costs >=2.2e-2 rel err alone (errors pass through
    attention averaging undiminished); tolerance is 2e-2.
"""

import os
import sys

import numpy as np

sys.path.insert(0, "/opt/trn_rl_repo")

import concourse.bass as bass  # noqa: E402
import concourse.mybir as mybir  # noqa: E402
from concourse import bacc, library_config  # noqa: E402
from concourse.tile import TileContext  # noqa: E402

P = 128
D = 512
S = 2048
HLOC = 4  # heads per core
DH = 64
NPANEL = 8
QP = 256  # query rows per panel
JW = HLOC * DH  # local j-width of K/Q/V projections (256)
SCALE = 0.125  # 1/sqrt(DH)

MMDT_NAME = os.environ.get("KERNEL_MMDT", "bf16")
MASK_GS = os.environ.get("KERNEL_MASK_GS", "0") == "1"
PVDEPTH = int(os.environ.get("KERNEL_PVDEPTH", "4"))
VCOPY_ACT = os.environ.get("KERNEL_VCOPY_ACT", "0") == "1"

f32 = mybir.dt.float32
Exp = mybir.ActivationFunctionType.Exp
add_op = mybir.AluOpType.add
mult_op = mybir.AluOpType.mult

MMDT = {"bf16": mybir.dt.bfloat16, "f32r": mybir.dt.float32r, "f32": f32}[MMDT_NAME]


def build():
    # Bacc (not Bass): its compile() pipeline runs generate_event_semaphores,
    # which splits multi-wait instructions to satisfy the 1-wait-per-
    # instruction hardware limit.
    nc = bacc.Bacc()

    xTp = nc.declare_dram_parameter("xTp", [P, 4, 4, 512], MMDT, isOutput=False)
    wkp = nc.declare_dram_parameter("wkp", [P, 4, JW], MMDT, isOutput=False)
    wqp = nc.declare_dram_parameter("wqp", [P, 4, JW], MMDT, isOutput=False)
    wvp = nc.declare_dram_parameter("wvp", [P, 4, JW], MMDT, isOutput=False)
    wop = nc.declare_dram_parameter("wop", [P, 2, D], MMDT, isOutput=False)
    bq = nc.declare_dram_parameter("bq", [P, 2], f32, isOutput=False)
    bo_bc = nc.declare_dram_parameter("bo_bc", [P, D], f32, isOutput=False)
    maskp = nc.declare_dram_parameter("maskp", [P, 2, 2, QP], MMDT, isOutput=False)
    # [panel, partition, q-subtile, d]: lets each panel's output leave in a
    # single contiguous dma_start (the host un-permutes)
    out = nc.declare_dram_parameter("out", [NPANEL, P, 2, D], MMDT, isOutput=True)

    with nc.allow_low_precision(reason="bf16 matmul operands"), TileContext(nc) as tc:
        with (
            tc.tile_pool(name="big", bufs=1) as bpool,
            tc.tile_pool(name="attp", bufs=2) as apool,
            tc.tile_pool(name="work", bufs=PVDEPTH + 2) as wpool,
            tc.tile_pool(name="osb", bufs=2) as opool,
            tc.tile_pool(name="ps_proj", bufs=2, space="PSUM") as ps_proj,
            tc.tile_pool(name="ps_s", bufs=2, space="PSUM") as ps_s,
            tc.tile_pool(name="ps_ot", bufs=2, space="PSUM") as ps_ot,
        ):
            # ---- persistent SBUF tensors ----
            xT_sb = bpool.tile([P, 4, 4, 512], MMDT, tag="xT")
            # K^T/Q^T head-PAIR-stacked: head 2pr on partitions 0-63,
            # head 2pr+1 on 64-127.
            kT_sb = bpool.tile([P, 2, S], MMDT, tag="kT")
            qT_sb = bpool.tile([P, 2, S], MMDT, tag="qT")
            v_sb = bpool.tile([P, S // P, HLOC, DH + 1], MMDT, tag="v")
            wk_sb = bpool.tile([P, 4, JW], MMDT, tag="wk", name="wk")
            wq_sb = bpool.tile([P, 4, JW], MMDT, tag="wq", name="wq")
            wv_sb = bpool.tile([P, 4, JW], MMDT, tag="wv", name="wv")
            wo_sb = bpool.tile([P, 2, D], MMDT, tag="wo")
            bq_sb = bpool.tile([P, 2], f32, tag="bq")
            bo_sb = bpool.tile([P, D], f32, tag="bo")
            mask_sb = bpool.tile([P, 2, 2, QP], MMDT, tag="mask")

            # input DMAs in consumption order; every transfer is contiguous.
            # Each dma_start costs ~0.6us of Sync-engine descriptor issue, so
            # transfers are batched as large as dependency order allows.
            nc.sync.dma_start(out=wk_sb[:], in_=wkp[:])
            nc.sync.dma_start(out=xT_sb[:, 0, 0:2], in_=xTp[:, 0, 0:2])
            nc.sync.dma_start(out=xT_sb[:, 0, 2:4], in_=xTp[:, 0, 2:4])
            nc.sync.dma_start(out=wq_sb[:], in_=wqp[:])
            nc.sync.dma_start(out=bq_sb[:], in_=bq[:])
            nc.sync.dma_start(out=mask_sb[:], in_=maskp[:])
            nc.sync.dma_start(out=wv_sb[:], in_=wvp[:])
            nc.sync.dma_start(out=xT_sb[:, 1], in_=xTp[:, 1])
            nc.sync.dma_start(out=wo_sb[:], in_=wop[:])
            nc.sync.dma_start(out=bo_sb[:], in_=bo_bc[:])
            nc.sync.dma_start(out=xT_sb[:, 2:4], in_=xTp[:, 2:4])
            # ones column appended per head so P.V also yields the softmax
            # denominator in psum row DH for free
            nc.vector.memset(v_sb[:, :, :, DH : DH + 1], 1.0)
            # gpsimd runs the 1/den partition broadcast (attn library)
            nc.gpsimd.load_library(library_config.attn)

            def proj_kq_gen(kc):
                """Yields after each matmul group so the caller can
                interleave projection work into the attention stream."""
                # K^T[j, k-chunk]; no bias (cancels in softmax)
                for pr in range(2):
                    ps = ps_proj.tile([P, 512], f32, tag="p512")
                    for db in range(4):
                        nc.tensor.matmul(
                            ps[:],
                            lhsT=wk_sb[:, db, pr * P : (pr + 1) * P],
                            rhs=xT_sb[:, kc, db, :],
                            start=(db == 0),
                            stop=(db == 3),
                        )
                    # pure cast -> ScalarE (DVE is the busier drain engine)
                    nc.scalar.copy(
                        out=kT_sb[:, pr, kc * 512 : (kc + 1) * 512], in_=ps[:]
                    )
                    yield
                # Q^T for q-chunk kc (panels 2kc, 2kc+1)
                for pr in range(2):
                    ps = ps_proj.tile([P, 512], f32, tag="p512")
                    for db in range(4):
                        nc.tensor.matmul(
                            ps[:],
                            lhsT=wq_sb[:, db, pr * P : (pr + 1) * P],
                            rhs=xT_sb[:, kc, db, :],
                            start=(db == 0),
                            stop=(db == 3),
                        )
                    nc.vector.tensor_tensor(
                        qT_sb[:, pr, kc * 512 : (kc + 1) * 512],
                        ps[:],
                        bq_sb[:, pr : pr + 1].to_broadcast([P, 512]),
                        add_op,
                    )
                    yield

            def proj_v_gen(kc):
                # V[k-chunk, j]; no bias (folded into bo' host-side)
                for kb in range(4):
                    ps = ps_proj.tile([P, 512], f32, tag="p512")
                    pv = ps[:, 0:JW]
                    for db in range(4):
                        nc.tensor.matmul(
                            pv,
                            lhsT=xT_sb[:, kc, db, kb * P : (kb + 1) * P],
                            rhs=wv_sb[:, db, :],
                            start=(db == 0),
                            stop=(db == 3),
                        )
                    # early chunks land while ACT is exp-light; later chunks
                    # drain on DVE to keep the exp stream unobstructed
                    if VCOPY_ACT or kc <= 1:
                        nc.scalar.copy(
                            out=v_sb[:, 4 * kc + kb, :, 0:DH],
                            in_=pv.rearrange("p (h d) -> p h d", h=HLOC),
                        )
                    else:
                        nc.vector.tensor_copy(
                            out=v_sb[:, 4 * kc + kb, :, 0:DH],
                            in_=pv.rearrange("p (h d) -> p h d", h=HLOC),
                        )
                    yield

            def make_norm(pr, ot_ps, attT_sb, split_hh=False, den_act=False):
                def emit_norm():
                    # attT[:, pr, :] = ot / den; den sits in psum row DH.
                    # custom-DVE recip must read SBUF (PSUM input returns
                    # garbage on HW even though CoreSim accepts it).
                    den_sb = wpool.tile([1, 2, QP], f32, tag="den_sb")
                    rden_f = wpool.tile([1, 2, QP], f32, tag="rden_f")
                    bc_sb = wpool.tile([DH, 2, QP], f32, tag="bc_sb")
                    if split_hh:
                        # per-hh chains pipeline across DVE/GpSimd: lower
                        # latency; den_act puts the PSUM drain on ScalarE
                        # (idle at the kernel tail) so DVE starts recip sooner
                        for hh in range(2):
                            if den_act:
                                nc.scalar.copy(
                                    out=den_sb[:, hh, :],
                                    in_=ot_ps[DH : DH + 1, hh, :],
                                )
                            else:
                                nc.vector.tensor_copy(
                                    out=den_sb[:, hh, :],
                                    in_=ot_ps[DH : DH + 1, hh, :],
                                )
                            nc.vector.reciprocal_approx_fast(
                                out=rden_f[:, hh, :], in_=den_sb[:, hh, :]
                            )
                            nc.gpsimd.partition_broadcast(
                                bc_sb[:, hh, :], rden_f[:, hh, :]
                            )
                    else:
                        nc.vector.tensor_copy(
                            out=den_sb[:], in_=ot_ps[DH : DH + 1, :, :]
                        )
                        nc.vector.reciprocal_approx_fast(
                            out=rden_f[:], in_=den_sb[:]
                        )
                        # broadcast 1/den across the 64 dh partitions on
                        # gpsimd (keeps PE/DVE out of the norm critical path)
                        nc.gpsimd.partition_broadcast(bc_sb[:], rden_f[:])
                    for hh in range(2):
                        nc.vector.tensor_mul(
                            out=attT_sb[hh * DH : (hh + 1) * DH, pr, :],
                            in0=ot_ps[0:DH, hh, :],
                            in1=bc_sb[:, hh, :],
                        )

                return emit_norm

            # last panel: O-projection accumulates per head pair as each
            # norm completes, so the kernel tail is one matmul pair
            last_ps = {}

            def make_o_mm(p, pr, attT_sb, start, stop):
                def emit():
                    osb = None
                    for qs in range(2):
                        if start:
                            last_ps[qs] = ps_proj.tile(
                                [P, D], f32, tag="p512", name=f"lastps{qs}"
                            )
                        nc.tensor.matmul(
                            last_ps[qs][:],
                            lhsT=attT_sb[:, pr, qs * P : (qs + 1) * P],
                            rhs=wo_sb[:, pr, :],
                            start=start,
                            stop=stop,
                        )
                        if stop:
                            if osb is None:
                                osb = opool.tile([P, 2, D], MMDT, tag="osb")
                            nc.vector.tensor_tensor(
                                osb[:, qs, :], last_ps[qs][:], bo_sb[:], add_op
                            )
                    if stop:
                        nc.sync.dma_start(out=out[p], in_=osb[:])

                return emit

            def make_oproj(p, attT_sb):
                def emit_oproj():
                    # out[q,:] = attT.T @ Wo_h.T + bo'; the two head pairs
                    # contract 128 partitions each
                    osb = opool.tile([P, 2, D], MMDT, tag="osb")
                    for qs in range(2):
                        ps = ps_proj.tile([P, D], f32, tag="p512")
                        for pr in range(2):
                            nc.tensor.matmul(
                                ps[:],
                                lhsT=attT_sb[:, pr, qs * P : (qs + 1) * P],
                                rhs=wo_sb[:, pr, :],
                                start=(pr == 0),
                                stop=(pr == 1),
                            )
                        nc.vector.tensor_tensor(
                            osb[:, qs, :], ps[:], bo_sb[:], add_op
                        )
                    nc.sync.dma_start(out=out[p], in_=osb[:])

                return emit_oproj

            # deferred work from panel p-1, emitted at staggered slots inside
            # panel p's batch stream so norm chains / O-projections hide
            # behind dense score+PV work. Entries are (min_slot, fn); the
            # DVE-only norms go early, the PE-visible O-proj late (its
            # matmuls wait on attT, and the in-order PE queue would
            # head-of-line block everything emitted after it).
            deferred_q = []
            # the final (masked) batch's PVs of panel p are CARRIED into
            # panel p+1's first slots, popped after each slot's exp: the PE
            # never idles at a panel boundary waiting on the exp->mask chain
            # of the triangular batch, and each pair's norm fires with its
            # carried PV.
            carry = []

            def emit_attention_panel(p, gen, drain_gen=True, rate=1):
                nbat = p + 1  # 2 k-blocks per exp batch, k < 256(p+1)
                q0 = p * QP
                attT_sb = apool.tile([P, 2, QP], MMDT, tag="attT")
                ot_ps = [
                    ps_ot.tile([DH + 1, 2, QP], f32, tag="ot", name=f"ot{pr}")
                    for pr in range(2)
                ]

                def emit_pv(bb, pr, pT):
                    for kbi in range(2):
                        for hh in range(2):
                            h = 2 * pr + hh
                            nc.tensor.matmul(
                                ot_ps[pr][:, hh, :],
                                lhsT=v_sb[:, 2 * bb + kbi, h, :],
                                rhs=pT[:, hh, kbi, :],
                                start=(bb == 0 and kbi == 0 and hh == 0),
                                stop=(bb == nbat - 1 and kbi == 1 and hh == 1),
                            )

                # the two head pairs' batches are INTERLEAVED (pr inner) so
                # each pair's norm chain hides behind the other pair's dense
                # score/PV stream; PV for slot s-PVDEPTH is emitted after
                # scores+exp of slot s, so the in-order PE queue always has
                # ready matmuls while ACT runs the exp
                # the LAST panel runs pr1 before pr0 each batch so pr1's
                # norm chain and O-matmuls overlap pr0's final PV stream
                pr_order = (1, 0) if p == NPANEL - 1 else (0, 1)
                pending = []
                slot = 0
                for bb in range(nbat):
                    for pr in pr_order:
                        s_ps = ps_s.tile([P, 2, 2, QP], f32, tag="s")
                        for kbi in range(2):
                            kb = 2 * bb + kbi
                            for hh in range(2):
                                nc.tensor.matmul(
                                    s_ps[:, hh, kbi, :],
                                    lhsT=kT_sb[
                                        hh * DH : (hh + 1) * DH,
                                        pr,
                                        kb * P : (kb + 1) * P,
                                    ],
                                    rhs=qT_sb[
                                        hh * DH : (hh + 1) * DH, pr, q0 : q0 + QP
                                    ],
                                    start=True,
                                    stop=True,
                                )
                        pT = wpool.tile([P, 2, 2, QP], MMDT, tag="pT")
                        nc.scalar.activation(pT[:], s_ps[:], Exp, scale=SCALE)
                        if bb == nbat - 1:
                            # zero masked probabilities in the 2 triangular
                            # k-blocks; mask pre-expanded over the head dim ->
                            # no broadcast operand -> DVE 2x mode
                            for kbi in range(2):
                                if MASK_GS:
                                    eng = nc.gpsimd if kbi == 1 else nc.vector
                                else:
                                    eng = nc.vector
                                eng.tensor_tensor(
                                    pT[:, :, kbi, :],
                                    pT[:, :, kbi, :],
                                    mask_sb[:, kbi, :, :],
                                    mult_op,
                                )
                        pending.append((bb, pr, pT))
                        while deferred_q and deferred_q[0][0] <= slot:
                            deferred_q.pop(0)[1]()
                        if carry:
                            fn, after = carry.pop(0)
                            fn()
                            after()
                        elif len(pending) > PVDEPTH:
                            emit_pv(*pending.pop(0))
                        # keep PE dense: pull next projection group for the
                        # following k-chunk while ACT digests this batch
                        if gen is not None and slot % rate == 0:
                            next(gen, None)
                        slot += 1
                last = p == NPANEL - 1
                if last:
                    # kernel tail: each pair's norm chain starts as soon as
                    # its last PV is emitted; pr1's O-matmuls then run on PE
                    # while DVE/GpSimd still normalize pr0
                    norms = {
                        pr: make_norm(
                            pr, ot_ps[pr], attT_sb, split_hh=True, den_act=True
                        )
                        for pr in range(2)
                    }
                    for i, item in enumerate(pending):
                        emit_pv(*item)
                        pr = item[1]
                        if all(it[1] != pr for it in pending[i + 1 :]):
                            norms.pop(pr)()
                    for pr in norms:
                        norms[pr]()
                    make_o_mm(p, pr_order[0], attT_sb, start=True, stop=False)()
                    make_o_mm(p, pr_order[1], attT_sb, start=False, stop=True)()
                else:
                    for item in pending:
                        if item[0] < nbat - 1:
                            emit_pv(*item)
                    carry.extend(
                        (
                            lambda item=item, emit_pv=emit_pv: emit_pv(*item),
                            make_norm(
                                item[1], ot_ps[item[1]], attT_sb, split_hh=True
                            ),
                        )
                        for item in pending
                        if item[0] == nbat - 1
                    )
                    while deferred_q:
                        deferred_q.pop(0)[1]()
                    deferred_q.append((5, make_oproj(p, attT_sb)))
                # drain any leftover projection groups of the next chunk
                if gen is not None and drain_gen:
                    for _ in gen:
                        pass

            def proj_chunk_gen(kc):
                yield from proj_kq_gen(kc)
                yield from proj_v_gen(kc)

            # chunk-0 K/Q go first (panel 0's scores need them); its V
            # groups interleave into panel 0 so the first exp starts sooner.
            # chunk 1 feeds panel 2 (Q) / panel 3 (K,V); chunks 2-3 are
            # spread across two panels each at half rate so the late panels'
            # boundaries still have independent PE work to hide norm chains
            for _ in proj_kq_gen(0):
                pass
            gens = {1: proj_chunk_gen(1), 2: proj_chunk_gen(2), 3: proj_chunk_gen(3)}
            sched = {
                0: (proj_v_gen(0), True, 1),
                1: (gens[1], True, 1),
                2: (gens[2], False, 2),
                3: (gens[2], True, 2),
                4: (gens[3], False, 2),
                5: (gens[3], True, 2),
            }
            for p in range(NPANEL):
                gen, drain, rate = sched.get(p, (None, True, 1))
                emit_attention_panel(p, gen, drain, rate)
    return nc


_NC = None


def _get_nc():
    global _NC
    if _NC is None:
        _NC = build()
        # run_bass_via_pjrt does not finalize; Bacc.finalize runs the compile
        # passes (register allocation, event-semaphore wait splitting).
        _NC.finalize()
    return _NC


def _mask_tri(mmnp):
    # triangular masks for the last 2 k-blocks of every panel; panel-
    # independent: block i valid where i*128 + r <= c
    m = np.empty((P, 2, 2, QP), np.float32)
    r = np.arange(P)[:, None]
    c = np.arange(QP)[None, :]
    for i in range(2):
        mi = np.where(i * P + r <= c, 1.0, 0.0)
        m[:, i, 0, :] = mi
        m[:, i, 1, :] = mi
    return np.ascontiguousarray(m).astype(mmnp)


def _in_maps(inputs):
    mmnp = mybir.dt.np(MMDT)
    x = np.asarray(inputs["x"], np.float32)
    woT = np.asarray(inputs["W_O_w"], np.float32).T  # [(h,dh), n]
    bo_full = np.asarray(inputs["W_O_b"], np.float32)
    bv_full = np.asarray(inputs["W_V_b"], np.float32)
    bq_full = np.asarray(inputs["W_Q_b"], np.float32)
    mask = _mask_tri(mmnp)
    halves = []
    for h in range(2):
        jsl = slice(JW * h, JW * (h + 1))
        # [P, db, j]: contraction row d = db*128 + p
        wq = np.ascontiguousarray(
            np.asarray(inputs["W_Q_w"], np.float32)[jsl].T.reshape(4, P, JW)
            .transpose(1, 0, 2)
        ).astype(mmnp)
        wk = np.ascontiguousarray(
            np.asarray(inputs["W_K_w"], np.float32)[jsl].T.reshape(4, P, JW)
            .transpose(1, 0, 2)
        ).astype(mmnp)
        wv = np.ascontiguousarray(
            np.asarray(inputs["W_V_w"], np.float32)[jsl].T.reshape(4, P, JW)
            .transpose(1, 0, 2)
        ).astype(mmnp)
        woT_h = woT[jsl]  # [256, 512]
        wo = np.ascontiguousarray(
            woT_h.reshape(2, P, D).transpose(1, 0, 2)
        ).astype(mmnp)
        bqh = np.ascontiguousarray(bq_full[jsl].reshape(2, P).T)
        # V bias folded through the O projection; bo itself only on half 0
        # (the two halves' partial outputs are summed on the host)
        bo_eff = (bo_full if h == 0 else 0.0) + bv_full[jsl] @ woT_h
        bo_bc = np.ascontiguousarray(np.broadcast_to(bo_eff, (P, D)))
        halves.append(dict(wqp=wq, wkp=wk, wvp=wv, wop=wo, bq=bqh, bo_bc=bo_bc))
    in_maps = []
    for core in range(8):
        b, h = core // 2, core % 2
        xT = x[b].T  # [D, S]
        xTp = np.ascontiguousarray(
            xT.reshape(4, P, 4, 512).transpose(1, 2, 0, 3)
        ).astype(mmnp)
        m = dict(halves[h])
        m["xTp"] = xTp
        m["maskp"] = mask
        in_maps.append(m)
    return in_maps


def _assemble(results, B=4):
    out = np.empty((B, S, D), np.float32)
    for b in range(B):
        # device layout [panel, p, qs, d] -> q = 256*panel + 128*qs + p
        s = results[2 * b]["out"].astype(np.float32) + results[2 * b + 1][
            "out"
        ].astype(np.float32)
        out[b] = s.reshape(NPANEL, P, 2, D).transpose(0, 2, 1, 3).reshape(S, D)
    return out


def run(inputs, trace=False, **kw):
    from concourse.bass_utils import run_bass_kernel_spmd

    res = run_bass_kernel_spmd(
        _get_nc(), _in_maps(inputs), core_ids=list(range(8)), trace=trace, **kw
    )
    return _assemble(res.results), res


def kernel(**inputs):
    out, _ = run(inputs, trace=False)
    return out
